# revision 1
# baseline (speedup 1.0000x reference)
"""DynamicSparseMoE grouped-GEMM kernel for 8 TRN2 NeuronCores.

out[t] = tokens[t] @ weight[exp_ids[t]]   (T=8192, E=8, D=2048 -> 2048)

Strategy (expert-parallel, host-side dispatch):
  - Host sorts tokens by expert; core e owns expert e's weight and its
    routed tokens, padded to a common capacity C (SPMD needs equal shapes).
  - Inputs are cast to fp16 on the host (PE runs fp16 at 1 cyc/row vs
    fp32's 4; PSUM accumulation stays fp32; rel-err ~3.6e-4 end to end).
  - Tokens are passed transposed ([D, C]): the stationary operand is a
    token block xT[d-block, 128 t] (one LDWEIGHTS per 4 matmuls), the
    moving operand is a weight slice w[d-block, 512 o], and PSUM gets
    out[t-block, o-slice] in the natural output orientation.
  - t-blocks run in pairs, contraction (kb) loop outermost inside the
    pair: 8 PSUM banks hold 2x4 accumulation groups. Pair 0 rides the
    startup DMA stream: the weight arrives as two half-width phases
    (wA = o 0-1023 per kb, then wB) so phase A is PE-bound against the
    half-rate stream; pair 0's stationary tokens come from a dedicated
    host-packed tile, its bulk on the Scalar DMA ring in parallel with
    the Sync ring's weight cadence. ~96 warm-up matmuls on a memset
    tile hold the HAM clock-gate open until real data lands.
  - The final partial block (<=64 real tokens) runs its four o-slices
    as two CONCURRENT column-group-packed matmul pairs (tile_position
    via PSUM base-partition 0/64, separate banks so start=True bank
    clears don't collide), halving its cost.
  - x and w are SBUF-resident (pair-streamed x fallback for extreme
    skew); output streams per t-block as fp16 on the Scalar ring and
    the host casts back to f32 and unpermutes.
"""

import os

import numpy as np

# A previously wedged NeuronCore (NRT_EXEC_UNIT_UNRECOVERABLE) recovers on
# the next init when core reset is requested; must be set before NRT init.
os.environ.setdefault("NEURON_RT_RESET_CORES", "1")

P = 128
D = 2048
E = 8
KB = D // P  # 16 contraction blocks
NOS = 4  # 4 moving slices of 512 over the 2048 output dim
NS = D // NOS  # 512

_cache = {}


def _ensure_imports():
    try:
        import concourse.bass  # noqa: F401
    except ImportError:
        import sys

        for p in ("/opt/trn_rl_repo", "/opt/pypackages"):
            if p not in sys.path:
                sys.path.append(p)


def _np_dt(compute_dt):
    if compute_dt == "float16":
        return np.float16
    import ml_dtypes

    return ml_dtypes.bfloat16


def _build(C, compute_dt="float16", last_m=128):
    """Build + compile the per-core Bass program for capacity C."""
    _ensure_imports()
    import concourse.bacc as bacc
    import concourse.mybir as mybir
    import concourse.tile as tile

    cdt = getattr(mybir.dt, compute_dt)
    TB = C // P  # t-blocks

    nc = bacc.Bacc(None, target_bir_lowering=False, debug=False)
    n0 = 2 * P if C // P >= 2 else P
    xt0_d = nc.declare_dram_parameter("xt0", [P, KB * n0], cdt, isOutput=False)
    xt_d = nc.declare_dram_parameter("xt", [D, C], cdt, isOutput=False)
    w_d = nc.declare_dram_parameter("w", [D, D], cdt, isOutput=False)
    out_d = nc.declare_dram_parameter("out", [C, D], cdt, isOutput=True)

    xt_t = xt_d.rearrange("(k p) n -> p k n", p=P)  # [128, 16, C]
    w_t = w_d.rearrange("(k p) o -> p k o", p=P)  # [128, 16, 2048]

    pairs = [[tb for tb in (p0, p0 + 1) if tb < TB] for p0 in range(0, TB, 2)]

    with tile.TileContext(nc) as tc:
        with (
            tc.tile_pool(name="wp", bufs=1) as wp,
            tc.tile_pool(name="xp", bufs=1) as xp,
            tc.tile_pool(name="op", bufs=3) as op,
            tc.tile_pool(name="pp", bufs=8, space="PSUM") as pp,
        ):
            # Pair 0's stationary blocks come from a dedicated host-packed
            # contiguous tile loaded before the 8 MB weight stream; the
            # remaining x arrives after the weights, by which time pairs 1+
            # still lead the PE comfortably. The very first matmul only
            # needs xp0's first kb-blocks and w0's first o-slice, so those
            # land as small separate DMAs ahead of everything else.
            # The weight streams as two half-width phases (os 0-1 then 2-3)
            # in separate tiles: pair 0 runs phase A PE-bound against the
            # half-rate A stream instead of dripping against full-width
            # per-kb arrivals, and phase B's data is resident by the time
            # A finishes.
            HD = D // 2  # 1024: columns per phase
            N0A = 2  # kb blocks in the first x chunk
            xp0a = xp.tile([P, N0A * n0], cdt, tag="xp0a")
            nc.sync.dma_start(xp0a[:], xt0_d[:, : N0A * n0])
            wA = [wp.tile([P, HD], cdt, tag=f"wA{kb}", name=f"wA{kb}") for kb in range(KB)]
            wB = [wp.tile([P, HD], cdt, tag=f"wB{kb}", name=f"wB{kb}") for kb in range(KB)]
            nc.sync.dma_start(wA[0][:, :NS], w_t[:, 0, :NS])
            nc.sync.dma_start(wA[0][:, NS:], w_t[:, 0, NS:HD])
            # xp0b rides the Scalar engine's HWDGE ring so it lands in
            # parallel with the uninterrupted wA cadence on the Sync ring.
            xp0b = xp.tile([P, (KB - N0A) * n0], cdt, tag="xp0b")
            nc.scalar.dma_start(xp0b[:], xt0_d[:, N0A * n0 :])
            for kb in range(1, KB):
                nc.sync.dma_start(wA[kb][:], w_t[:, kb, :HD])
            for kb in range(KB):
                nc.sync.dma_start(wB[kb][:], w_t[:, kb, HD:])

            def xp0(kb):
                if kb < N0A:
                    return xp0a[:, kb * n0 : (kb + 1) * n0]
                return xp0b[:, (kb - N0A) * n0 : (kb - N0A + 1) * n0]

            def w_slice(kb, os):
                if os < 2:
                    return wA[kb][:, os * NS : (os + 1) * NS]
                return wB[kb][:, (os - 2) * NS : (os - 1) * NS]
            # x fully SBUF-resident for normal capacities; for extreme expert
            # skew (C > 2944 would overflow SBUF) stream x per t-block pair.
            resident = C <= 2944
            if resident:
                x_sb = []
                for kb in range(KB):
                    xt_k = xp.tile([P, C], cdt, tag=f"x{kb}")
                    nc.sync.dma_start(xt_k[:], xt_t[:, kb, :])
                    x_sb.append(xt_k)
            else:
                x_pair = {}
                for pi in range(1, len(pairs)):
                    tbs = pairs[pi]
                    n = len(tbs) * P
                    t0 = tbs[0] * P
                    xpi = xp.tile(
                        [P, KB * n], cdt, tag="xpair", bufs=3, name=f"xpair{pi}"
                    )
                    nc.sync.dma_start(
                        xpi.rearrange("p (k n) -> p k n", k=KB),
                        xt_t[:, :, t0 : t0 + n],
                    )
                    x_pair[pi] = xpi

            def lhs(pi, kb, tb, ti, ntb):
                if pi == 0:
                    return xp0(kb)[:, ti * P : (ti + 1) * P]
                if resident:
                    return x_sb[kb][:, tb * P : (tb + 1) * P]
                return x_pair[pi][:, (kb * ntb + ti) * P : (kb * ntb + ti + 1) * P]

            # PE pre-warm: HAM keeps the PE clock-gated at 1.2 GHz until it
            # has seen ~3.4 us of sustained activity. Run dummy matmuls on
            # memset data during the initial DMA wait so the real matmuls
            # start at 2.4 GHz. They scribble on pair 0's first PSUM bank,
            # which the first real start=True matmul clears anyway.
            warm = xp.tile([P, 64], cdt, tag="warm")
            nc.vector.memset(warm[:], 0.0)

            for pi, tbs in enumerate(pairs):
                last = pi == len(pairs) - 1
                ps = {
                    (ti, os): pp.tile(
                        [P, NS], mybir.dt.float32, tag="ps", name=f"ps_{pi}_{ti}_{os}"
                    )
                    for ti in range(len(tbs))
                    for os in range(NOS)
                }
                if pi == 0:
                    for _ in range(96):
                        nc.tensor.matmul(
                            ps[(0, 0)][:64, :64],
                            lhsT=warm[:, :64],
                            rhs=warm[:, :64],
                            start=True,
                            stop=True,
                        )
                if last and len(tbs) == 1 and last_m == 64:
                    # Packed final block: the real tokens fit in 64 stationary
                    # columns, so run os pairs (0,1) and (2,3) CONCURRENTLY in
                    # the PE array's two column-group halves (tile_position
                    # auto-derived from the output base partition). Odd os
                    # groups land on partitions 64-127 of their own PSUM bank
                    # (separate banks, so start=True bank-clears don't collide).
                    H = P // 2
                    tb = tbs[0]
                    for kb in range(KB):
                        for os in range(NOS):
                            dst = (
                                ps[(0, os)][:H, :]
                                if os % 2 == 0
                                else ps[(0, os)][H:, :]
                            )
                            nc.tensor.matmul(
                                dst,
                                lhsT=lhs(pi, kb, tb, 0, 1)[:, :H],
                                rhs=w_slice(kb, os),
                                start=(kb == 0),
                                stop=(kb == KB - 1),
                            )
                    o_sb = op.tile([P, D], cdt, tag="o", name=f"oq_{pi}")
                    for os in range(NOS):
                        rows = slice(0, H) if os % 2 == 0 else slice(H, P)
                        nc.vector.tensor_copy(
                            o_sb[rows, os * NS : (os + 1) * NS],
                            ps[(0, os)][rows, :],
                        )
                        # Sync ring: idle at kernel end, while Scalar may
                        # still be draining the previous pair's 2 MB of output
                        nc.sync.dma_start(
                            out_d[tb * P : tb * P + H, os * NS : (os + 1) * NS],
                            o_sb[rows, os * NS : (os + 1) * NS],
                        )
                    continue
                # pair 0 follows the two-phase weight stream (os 0-1 while
                # the A halves land, then os 2-3); later pairs interleave
                # all four os per kb for 4-matmul LDWEIGHTS amortization.
                os_phases = [(0, 1), (2, 3)] if pi == 0 else [(0, 1, 2, 3)]
                for phase in os_phases:
                    for kb in range(KB):
                        for ti, tb in enumerate(tbs):
                            for os in phase:
                                nc.tensor.matmul(
                                    ps[(ti, os)][:],
                                    lhsT=lhs(pi, kb, tb, ti, len(tbs)),
                                    rhs=w_slice(kb, os),
                                    start=(kb == 0),
                                    stop=(kb == KB - 1),
                                )
                for ti, tb in enumerate(tbs):
                    o_sb = op.tile([P, D], cdt, tag="o", name=f"o_{pi}_{ti}")
                    for os in range(NOS):
                        nc.vector.tensor_copy(
                            o_sb[:, os * NS : (os + 1) * NS], ps[(ti, os)][:]
                        )
                        if last:
                            # tail: stream each 512-slice out as soon as its
                            # copy lands instead of one 1 MB DMA at the end
                            nc.scalar.dma_start(
                                out_d[tb * P : (tb + 1) * P, os * NS : (os + 1) * NS],
                                o_sb[:, os * NS : (os + 1) * NS],
                            )
                    if not last:
                        nc.scalar.dma_start(out_d[tb * P : (tb + 1) * P, :], o_sb[:])
    nc.compile()
    return nc


def _get_nc(C, compute_dt, last_m):
    key = (C, compute_dt, last_m)
    if key not in _cache:
        _cache[key] = _build(C, compute_dt, last_m)
    return _cache[key]


def kernel(tokens, weight, exp_ids, _trace=False, _compute_dt="float16"):
    _ensure_imports()
    from concourse.bass_utils import run_bass_kernel_spmd

    tokens = np.asarray(tokens)
    weight = np.asarray(weight)
    exp_ids = np.asarray(exp_ids)
    T = tokens.shape[0]

    order = np.argsort(exp_ids, kind="stable")
    counts = np.bincount(exp_ids, minlength=E)
    C = max(int(-(-counts.max() // P) * P), NS)

    starts = np.zeros(E + 1, dtype=np.int64)
    np.cumsum(counts, out=starts[1:])

    # Packed final block is valid when the last 128-block holds <= 64 real
    # tokens on every core and the block count is odd (lone final block).
    TB = C // P
    rest = int(counts.max()) - (TB - 1) * P
    last_m = 64 if (TB >= 3 and TB % 2 == 1 and rest <= 64) else 128

    npdt = _np_dt(_compute_dt)
    tokens_c = tokens.astype(npdt)
    weight_c = weight.astype(npdt)

    n0 = 2 * P if C // P >= 2 else P
    in_maps = []
    for e in range(E):
        idx = order[starts[e] : starts[e + 1]]
        xt = np.zeros((D, C), dtype=npdt)
        xt[:, : counts[e]] = tokens_c[idx].T
        # xt0: first-pair stationary blocks packed [p, kb*n0 + t] contiguously
        xt0 = np.ascontiguousarray(
            xt[:, :n0].reshape(KB, P, n0).transpose(1, 0, 2).reshape(P, KB * n0)
        )
        in_maps.append({"xt": xt, "xt0": xt0, "w": np.ascontiguousarray(weight_c[e])})

    nc = _get_nc(C, _compute_dt, last_m)
    res = run_bass_kernel_spmd(
        nc,
        in_maps,
        core_ids=list(range(E)),
        trace=_trace,
        trace_cores=list(range(E)) if _trace else None,
    )

    out = np.empty((T, D), dtype=np.float32)
    for e in range(E):
        idx = order[starts[e] : starts[e + 1]]
        out[idx] = res.results[e]["out"][: counts[e], :].astype(np.float32)
    if _trace:
        return out, res
    return out



# revision 5
# speedup vs baseline: 1.0158x; 1.0158x over previous
"""DynamicSparseMoE grouped-GEMM kernel for 8 TRN2 NeuronCores — raw bass.

out[t] = tokens[t] @ weight[exp_ids[t]]   (T=8192, E=8, D=2048 -> 2048)

Strategy (expert-parallel, host-side dispatch, same math as the Tile
baseline but emitted as a raw bass program with ~12 semaphores):
  - Host sorts tokens by expert; core e owns expert e's weight and its
    routed tokens, padded to a common capacity C (SPMD needs equal shapes).
  - fp16 compute (PE 1 cyc/row), fp32 PSUM accumulation.
  - Tokens are the stationary operand (xT blocks [128 d, 128 t]); the
    weight is the moving operand in 512-wide o-slices.
  - DRAM input layouts mirror SBUF exactly (host pre-packs), so every
    DMA is one contiguous chunked copy on a HWDGE ring:
      sync ring:   x(pair0 first 2 kb) | wA x8 | x(block2) | wB x8 | x(blocks 3..)
      scalar ring: x(pair0 kb2-15, 2 chunks) | per-block output stores
  - Blocks 0,1 run kb-outer in two half-width weight phases (A = o 0-1023,
    B = o 1024-2047) so the PE chases the half-rate weight stream without
    stalling. Blocks 2+ run per-(block, o-slice) units with kb innermost:
    each unit accumulates one PSUM bank over 16 kb steps, then the bank is
    copied out and its 512-slice streams to DRAM while later units compute
    (continuous output drain, rolling over all 8 PSUM banks).
  - The final partial block (<=64 real tokens) runs its o-slices as two
    column-group-packed concurrent matmul pairs (PSUM base partition 0/64).
  - ~36 warmup matmuls on a memset tile keep the HAM clock-gate open
    between the fixed ~6us program prologue and first weight arrival.
  - All semaphores are cleared at program end (the NEFF is executed more
    than once per session; sems must return to 0).

Relative to the Tile-framework baseline this removes the ~10us
semaphore-clear epilogue (~236 sems -> 12) and starts the real matmul
stream several us earlier via explicit DMA choreography.
"""

import os
from contextlib import ExitStack

import numpy as np

# A previously wedged NeuronCore (NRT_EXEC_UNIT_UNRECOVERABLE) recovers on
# the next init when core reset is requested; must be set before NRT init.
os.environ.setdefault("NEURON_RT_RESET_CORES", "1")

P = 128
D = 2048
E = 8
KB = D // P  # 16 contraction blocks
HD = D // 2  # 1024: columns per weight phase
NS = 512  # o-slice width (one PSUM bank)
N_WARM = 36

_cache = {}


def _ensure_imports():
    try:
        import concourse.bass  # noqa: F401
    except ImportError:
        import sys

        for p in ("/opt/trn_rl_repo", "/opt/pypackages"):
            if p not in sys.path:
                sys.path.append(p)


def _build(C, last_m=128):
    """Build + compile the per-core raw-bass program for capacity C."""
    _ensure_imports()
    import concourse.bacc as bacc
    import concourse.mybir as mybir

    f16 = mybir.dt.float16
    f32 = mybir.dt.float32
    TB = C // P
    assert TB >= 4, C
    NB = TB - 2  # blocks handled per-block (indices 2..TB-1)
    packed = last_m == 64

    nc = bacc.Bacc(None, target_bir_lowering=False, debug=False)

    xs_d = nc.declare_dram_parameter("xs", [P, KB * C], f16, isOutput=False)
    wa_d = nc.declare_dram_parameter("wa", [P, KB * HD], f16, isOutput=False)
    wb_d = nc.declare_dram_parameter("wb", [P, KB * HD], f16, isOutput=False)
    out_d = nc.declare_dram_parameter("out", [C, D], f16, isOutput=True)

    # xs column offset of the stationary block (b, kb); pair0 is kb-major
    # with 256 token-columns per kb, blocks 2+ are block-major.
    def xoff(b, kb):
        if b < 2:
            return kb * 256 + b * P
        return KB * 256 + (b - 2) * KB * P + kb * P

    es = ExitStack()
    with es:
        xs = es.enter_context(nc.sbuf_tensor("xs_sb", [P, KB * C], f16))
        wa = es.enter_context(nc.sbuf_tensor("wa_sb", [P, KB * HD], f16))
        wb = es.enter_context(nc.sbuf_tensor("wb_sb", [P, KB * HD], f16))
        o_sb = es.enter_context(nc.sbuf_tensor("o_sb", [P, 3 * D], f16))
        warm = es.enter_context(nc.sbuf_tensor("warm", [P, 64], f16))
        ps = [
            es.enter_context(nc.psum_tensor(f"ps{i}", [P, NS], f32)) for i in range(8)
        ]
        sems = {}
        for name in (
            "ws",  # vector memset -> tensor warmups
            "x0a",  # sync: pair0 kb0-1 x
            "x0b",  # scalar: pair0 kb2-15 x (2 chunks)
            "xb",  # sync: per-block x for blocks 2+
            "wa",  # sync: wA chunks (8)
            "wb",  # sync: wB chunks (8)
            "mm",  # tensor unit done -> vector
            "cp",  # vector unit copies done -> tensor bank reuse
            "co",  # vector block output assembled -> scalar store
            "cf",  # vector final-block copies -> sync store
            "osc",  # scalar out-DMA completions
            "osy",  # sync out-DMA completions (packed final block)
        ):
            sems[name] = es.enter_context(nc.semaphore(f"s_{name}"))

        # ---- unit table (shared by tensor and vector emission) ----
        # unit: (kind, block, os, banks)
        units = []
        units.append(("ph", 0, (0, 1), [0, 1, 2, 3]))  # phase A: blocks 0,1
        units.append(("ph", 0, (2, 3), [4, 5, 6, 7]))  # phase B
        bank_cursor = 0
        last_use = {i: u for u, bs in ((0, [0, 1, 2, 3]), (1, [4, 5, 6, 7])) for i in bs}
        for b in range(2, TB):
            if packed and b == TB - 1:
                units.append(("pf", b, (0, 1), None))
                units.append(("pf", b, (2, 3), None))
            else:
                for osl in range(4):
                    units.append(("bl", b, osl, None))
        for i, (kind, b, osl, banks) in enumerate(units):
            if banks is None:
                n = 2 if kind == "pf" else 1
                banks = [(bank_cursor + j) % 8 for j in range(n)]
                bank_cursor += n
                units[i] = (kind, b, osl, banks)

        def w_slice(kb, osl):
            t = wa if osl < 2 else wb
            return t[:, kb * HD + (osl % 2) * NS : kb * HD + (osl % 2) * NS + NS]

        # ---------------- sync engine: input stream + final stores ----
        sy = nc.sync
        sy.dma_start(xs[:, :512], xs_d[:, :512]).then_inc(sems["x0a"], 16)
        for c in range(8):
            sl = slice(c * 2 * HD, (c + 1) * 2 * HD)
            sy.dma_start(wa[:, sl], wa_d[:, sl]).then_inc(sems["wa"], 16)
        nxb = 0
        if TB > 2:
            sl = slice(xoff(2, 0), xoff(2, 0) + KB * P)
            sy.dma_start(xs[:, sl], xs_d[:, sl]).then_inc(sems["xb"], 16)
            nxb = 1
        for c in range(8):
            sl = slice(c * 2 * HD, (c + 1) * 2 * HD)
            sy.dma_start(wb[:, sl], wb_d[:, sl]).then_inc(sems["wb"], 16)
        for b in range(3, TB):
            sl = slice(xoff(b, 0), xoff(b, 0) + KB * P)
            sy.dma_start(xs[:, sl], xs_d[:, sl]).then_inc(sems["xb"], 16)
            nxb += 1
        if packed:
            tbf = TB - 1
            fbuf = (tbf % 3) * D
            for osl in range(4):
                rows = slice(0, 64) if osl % 2 == 0 else slice(64, P)
                sy.wait_ge(sems["cf"], 1 + osl // 2)
                sy.dma_start(
                    out_d[tbf * P : tbf * P + 64, osl * NS : (osl + 1) * NS],
                    o_sb[rows, fbuf + osl * NS : fbuf + (osl + 1) * NS],
                ).then_inc(sems["osy"], 16)
            sy.wait_ge(sems["osy"], 64)

        # ---------------- scalar engine: pair0 x tail + output stores --
        sc = nc.scalar
        sc.dma_start(xs[:, 512:2048], xs_d[:, 512:2048]).then_inc(sems["x0b"], 16)
        sc.dma_start(xs[:, 2048:4096], xs_d[:, 2048:4096]).then_inc(sems["x0b"], 16)
        n_out = TB - 1 if packed else TB
        for b in range(n_out):
            sc.wait_ge(sems["co"], b + 1)
            buf = (b % 3) * D
            sc.dma_start(
                out_d[b * P : (b + 1) * P, :], o_sb[:, buf : buf + D]
            ).then_inc(sems["osc"], 16)
        sc.wait_ge(sems["osc"], 16 * n_out)

        # ---------------- vector engine: memset + PSUM->SBUF copies ----
        ve = nc.vector
        ve.memset(warm[:], 0.0).then_inc(sems["ws"], 1)
        for u, (kind, b, osl, banks) in enumerate(units):
            ve.wait_ge(sems["mm"], u + 1)
            if kind == "ph":
                for ti in range(2):
                    buf = ((b + ti) % 3) * D
                    for j, o in enumerate(osl):
                        ve.tensor_copy(
                            o_sb[:, buf + o * NS : buf + (o + 1) * NS],
                            ps[banks[2 * ti + j]][:],
                        )
                if osl[0] == 2:
                    # phase B completes both blocks 0 and 1
                    ve.nop().then_inc(sems["co"], 2)
            elif kind == "bl":
                if osl == 0 and b >= 3:
                    ve.wait_ge(sems["osc"], 16 * (b - 2))
                buf = (b % 3) * D
                ve.tensor_copy(
                    o_sb[:, buf + osl * NS : buf + (osl + 1) * NS], ps[banks[0]][:]
                )
                if osl == 3:
                    ve.nop().then_inc(sems["co"], 1)
            else:  # packed final
                if osl[0] == 0 and b >= 3:
                    ve.wait_ge(sems["osc"], 16 * (b - 2))
                buf = (b % 3) * D
                for j, o in enumerate(osl):
                    rows = slice(0, 64) if o % 2 == 0 else slice(64, P)
                    ve.tensor_copy(
                        o_sb[rows, buf + o * NS : buf + (o + 1) * NS],
                        ps[banks[j]][rows, :],
                    )
                ve.nop().then_inc(sems["cf"], 1)
            ve.nop().then_inc(sems["cp"], 1)

        # ---------------- tensor engine: warmups + matmul stream -------
        te = nc.tensor
        te.wait_ge(sems["ws"], 1)
        for _ in range(N_WARM):
            te.matmul(
                ps[0][:64, :64], lhsT=warm[:, :64], rhs=warm[:, :64],
                start=True, stop=True,
            )
        for u, (kind, b, osl, banks) in enumerate(units):
            # PSUM bank reuse: wait for the copy of the unit that last
            # used these banks.
            need = 0
            for bk in banks:
                if bk in last_use and u >= 2:
                    need = max(need, last_use[bk] + 1)
            if u >= 2:
                for bk in banks:
                    last_use[bk] = u
            if need:
                te.wait_ge(sems["cp"], need)
            if kind == "ph":
                wsem = sems["wa"] if osl[0] == 0 else sems["wb"]
                for kb in range(KB):
                    if kb % 2 == 0:
                        te.wait_ge(wsem, 16 * (kb // 2 + 1))
                    if osl[0] == 0:
                        if kb == 0:
                            te.wait_ge(sems["x0a"], 16)
                        elif kb == 2:
                            te.wait_ge(sems["x0b"], 16)
                        elif kb == 8:
                            te.wait_ge(sems["x0b"], 32)
                    for ti in range(2):
                        for j, o in enumerate(osl):
                            mm = te.matmul(
                                ps[banks[2 * ti + j]][:],
                                lhsT=xs[:, xoff(ti, kb) : xoff(ti, kb) + P],
                                rhs=w_slice(kb, o),
                                start=(kb == 0),
                                stop=(kb == KB - 1),
                            )
                mm.then_inc(sems["mm"], 1)
            elif kind == "bl":
                if osl == 0:
                    te.wait_ge(sems["xb"], 16 * (b - 1))
                    if b == 2:
                        te.wait_ge(sems["wa"], 128)
                        te.wait_ge(sems["wb"], 128)
                for kb in range(KB):
                    mm = te.matmul(
                        ps[banks[0]][:],
                        lhsT=xs[:, xoff(b, kb) : xoff(b, kb) + P],
                        rhs=w_slice(kb, osl),
                        start=(kb == 0),
                        stop=(kb == KB - 1),
                    )
                mm.then_inc(sems["mm"], 1)
            else:  # packed final: two concurrent column-group halves
                if osl[0] == 0:
                    te.wait_ge(sems["xb"], 16 * (b - 1))
                for kb in range(KB):
                    for j, o in enumerate(osl):
                        dst = ps[banks[j]][:64, :] if o % 2 == 0 else ps[banks[j]][64:, :]
                        mm = te.matmul(
                            dst,
                            lhsT=xs[:, xoff(b, kb) : xoff(b, kb) + 64],
                            rhs=w_slice(kb, o),
                            start=(kb == 0),
                            stop=(kb == KB - 1),
                        )
                mm.then_inc(sems["mm"], 1)

        # ---------------- teardown: reset sems for the next execution --
        nc.all_engine_barrier()
        for s in sems.values():
            nc.gpsimd.sem_clear(s)
        nc.all_engine_barrier()

        nc.compile()
    return nc


def _get_nc(C, last_m):
    key = (C, last_m)
    if key not in _cache:
        _cache[key] = _build(C, last_m)
    return _cache[key]


def kernel(tokens, weight, exp_ids, _trace=False):
    _ensure_imports()
    from concourse.bass_utils import run_bass_kernel_spmd

    tokens = np.asarray(tokens)
    weight = np.asarray(weight)
    exp_ids = np.asarray(exp_ids)
    T = tokens.shape[0]

    order = np.argsort(exp_ids, kind="stable")
    counts = np.bincount(exp_ids, minlength=E)
    C = max(int(-(-counts.max() // P) * P), 512)
    TB = C // P

    # Packed final block when the last 128-block holds <= 64 real tokens on
    # every core (the block count need not be odd here — blocks 2+ are
    # emitted singly).
    rest = int(counts.max()) - (TB - 1) * P
    last_m = 64 if (TB >= 4 and rest <= 64) else 128

    starts = np.zeros(E + 1, dtype=np.int64)
    np.cumsum(counts, out=starts[1:])

    tokens_c = tokens.astype(np.float16)
    weight_c = weight.astype(np.float16)

    in_maps = []
    for e in range(E):
        idx = order[starts[e] : starts[e + 1]]
        xt = np.zeros((D, C), dtype=np.float16)
        xt[:, : counts[e]] = tokens_c[idx].T
        xt3 = xt.reshape(KB, P, C)
        parts = [np.ascontiguousarray(xt3[:, :, :256].transpose(1, 0, 2)).reshape(P, -1)]
        for b in range(2, TB):
            parts.append(
                np.ascontiguousarray(
                    xt3[:, :, b * P : (b + 1) * P].transpose(1, 0, 2)
                ).reshape(P, -1)
            )
        xs = np.concatenate(parts, axis=1)
        w3 = weight_c[e].reshape(KB, P, D)
        wa = np.ascontiguousarray(w3[:, :, :HD].transpose(1, 0, 2)).reshape(P, -1)
        wb = np.ascontiguousarray(w3[:, :, HD:].transpose(1, 0, 2)).reshape(P, -1)
        in_maps.append({"xs": xs, "wa": wa, "wb": wb})

    nc = _get_nc(C, last_m)
    res = run_bass_kernel_spmd(
        nc,
        in_maps,
        core_ids=list(range(E)),
        trace=_trace,
        trace_cores=list(range(E)) if _trace else None,
    )

    out = np.empty((T, D), dtype=np.float32)
    for e in range(E):
        idx = order[starts[e] : starts[e + 1]]
        out[idx] = res.results[e]["out"][: counts[e], :].astype(np.float32)
    if _trace:
        return out, res
    return out


# revision 7
# speedup vs baseline: 1.0230x; 1.0070x over previous
"""DynamicSparseMoE grouped-GEMM kernel for 8 TRN2 NeuronCores — raw bass.

out[t] = tokens[t] @ weight[exp_ids[t]]   (T=8192, E=8, D=2048 -> 2048)

Strategy (expert-parallel, host-side dispatch):
  - Host sorts tokens by expert; core e owns expert e's weight and its
    routed tokens, padded to a common capacity C (SPMD needs equal shapes).
  - fp16 compute (PE 1 cyc/row), fp32 PSUM accumulation.
  - Tokens are the stationary operand (xT blocks [128 d, 128 t]); the
    weight is the moving operand in 512-wide o-slices.
  - DRAM input layouts mirror SBUF exactly (host pre-packs), so every
    DMA is one contiguous chunked copy on a HWDGE ring.
  - Startup: the early HBM stream ramps slowly (~150-300 GB/s for the
    first ~6 us), so blocks 0-3 run as a quad with os-slice-major phases
    (one 512-wide o-slice across 4 stationary blocks per phase, kb
    outermost): weight demand is 128 KB per 853 ns sweep (~150 GB/s),
    which the ramp can feed; the weight DRAM layout is os-major so the
    stream is consumed strictly in order.  ~44 warmup matmuls on a
    memset tile bridge the fixed ~7 us program prologue until the first
    weight chunk lands.
  - The packed final block (<=64 real tokens, two column-group-packed
    concurrent matmul pairs via PSUM base partition 0/64) runs right
    after the quad so the kernel does not end on it; remaining blocks
    run as per-(block, o-slice) units with kb innermost: each unit
    accumulates one PSUM bank, which is copied out and streamed to DRAM
    while later units compute (continuous output drain; the last block
    streams per-o-slice so the final transfer is only 128 KB).
  - All semaphores are cleared at program end (the NEFF is executed more
    than once per session; sems must return to 0).
"""

import os
from contextlib import ExitStack

import numpy as np

# A previously wedged NeuronCore (NRT_EXEC_UNIT_UNRECOVERABLE) recovers on
# the next init when core reset is requested; must be set before NRT init.
os.environ.setdefault("NEURON_RT_RESET_CORES", "1")

P = 128
D = 2048
E = 8
KB = D // P  # 16 contraction blocks
NS = 512  # o-slice width (one PSUM bank)
NOS = 4
N_WARM = 44

_cache = {}


def _ensure_imports():
    try:
        import concourse.bass  # noqa: F401
    except ImportError:
        import sys

        for p in ("/opt/trn_rl_repo", "/opt/pypackages"):
            if p not in sys.path:
                sys.path.append(p)


def _build(C, last_m=128):
    """Build + compile the per-core raw-bass program for capacity C."""
    _ensure_imports()
    import concourse.bacc as bacc
    import concourse.mybir as mybir

    f16 = mybir.dt.float16
    f32 = mybir.dt.float32
    TB = C // P
    assert TB >= 5, C
    packed = last_m == 64
    FB = TB - 1 if packed else None  # packed final block index
    blocks = [b for b in range(4, TB) if b != FB]  # regular per-block units
    QX = KB * 512  # xs columns of the quad (blocks 0-3)

    nc = bacc.Bacc(None, target_bir_lowering=False, debug=False)

    xs_d = nc.declare_dram_parameter("xs", [P, KB * C], f16, isOutput=False)
    w_d = nc.declare_dram_parameter("w", [P, NOS * KB * NS], f16, isOutput=False)
    out_d = nc.declare_dram_parameter("out", [C, D], f16, isOutput=True)

    # xs column offset of stationary block (b, kb); the quad (blocks 0-3)
    # is kb-major with 512 token-columns per kb, blocks 4+ block-major in
    # DMA order (packed final block first).
    border = ([FB] if packed else []) + blocks

    def xoff(b, kb):
        if b < 4:
            return kb * 512 + b * P
        return QX + border.index(b) * KB * P + kb * P

    def w_sl(t, osl, kb):
        off = (osl * KB + kb) * NS
        return t[:, off : off + NS]

    es = ExitStack()
    with es:
        xs = es.enter_context(nc.sbuf_tensor("xs_sb", [P, KB * C], f16))
        wt = es.enter_context(nc.sbuf_tensor("w_sb", [P, NOS * KB * NS], f16))
        o_sb = es.enter_context(nc.sbuf_tensor("o_sb", [P, 6 * D], f16))
        warm = es.enter_context(nc.sbuf_tensor("warm", [P, 64], f16))
        ps = [
            es.enter_context(nc.psum_tensor(f"ps{i}", [P, NS], f32)) for i in range(8)
        ]
        sems = {}
        for name in (
            "ws",  # vector memset -> tensor warmups
            "xqa",  # sync: quad x kb0-7 (4 chunks)
            "xqb",  # scalar: quad x kb8-15 (4 chunks)
            "xb",  # sync: per-block x for blocks 4+ (F first)
            "w",  # sync: weight chunks, os-major (16)
            "mm",  # tensor unit done -> vector
            "cp",  # vector unit copies done -> tensor bank reuse
            "co",  # vector output chunk assembled -> scalar store
            "cf",  # vector final-block copies -> sync store
            "osc",  # scalar out-DMA completions
            "osy",  # sync out-DMA completions (packed final block)
        ):
            sems[name] = es.enter_context(nc.semaphore(f"s_{name}"))

        # ---- unit table ----
        # ("qp", phase_os, banks) | ("pf", b, (os,os), banks) | ("bl", b, os, bank)
        units = []
        for p in range(NOS):
            units.append(("qp", None, p, [(p % 2) * 4 + i for i in range(4)]))
        if packed:
            units.append(("pf", FB, (0, 1), [0, 1]))
            units.append(("pf", FB, (2, 3), [2, 3]))
        bank_cursor = 4 if packed else 0
        for b in blocks:
            for osl in range(NOS):
                units.append(("bl", b, osl, [bank_cursor % 8]))
                bank_cursor += 1
        last_block = blocks[-1]

        # o_sb buffer per block: quad -> 0-3, packed final -> 5,
        # regular blocks cycle {0,1,2,4} (reuse gated on osc).
        def obuf(b):
            if b < 4:
                return b
            if b == FB:
                return 5
            return [0, 1, 2, 4][blocks.index(b) % 4]

        # ---------------- sync engine ---------------------------------
        sy = nc.sync
        # quad x kb0-7 interleaved with os0 weight chunks (4 kb each)
        for c in range(4):
            xsl = slice(c * 1024, (c + 1) * 1024)
            sy.dma_start(xs[:, xsl], xs_d[:, xsl]).then_inc(sems["xqa"], 16)
            wsl = slice(c * 4 * NS, (c + 1) * 4 * NS)
            sy.dma_start(wt[:, wsl], w_d[:, wsl]).then_inc(sems["w"], 16)
        for osl in range(1, NOS):
            for c in range(4):
                wsl = slice((osl * KB + c * 4) * NS, (osl * KB + (c + 1) * 4) * NS)
                sy.dma_start(wt[:, wsl], w_d[:, wsl]).then_inc(sems["w"], 16)
        for i, b in enumerate(border):
            xsl = slice(QX + i * KB * P, QX + (i + 1) * KB * P)
            sy.dma_start(xs[:, xsl], xs_d[:, xsl]).then_inc(sems["xb"], 16)
        if packed:
            fbuf = obuf(FB) * D
            for osl in range(NOS):
                rows = slice(0, 64) if osl % 2 == 0 else slice(64, P)
                sy.wait_ge(sems["cf"], 1 + osl // 2)
                sy.dma_start(
                    out_d[FB * P : FB * P + 64, osl * NS : (osl + 1) * NS],
                    o_sb[rows, fbuf + osl * NS : fbuf + (osl + 1) * NS],
                ).then_inc(sems["osy"], 16)
            sy.wait_ge(sems["osy"], 64)

        # ---------------- scalar engine: quad x kb8-15 + output stores -
        sc = nc.scalar
        for c in range(4, 8):
            xsl = slice(c * 1024, (c + 1) * 1024)
            sc.dma_start(xs[:, xsl], xs_d[:, xsl]).then_inc(sems["xqb"], 16)
        co_thr = 0
        n_osc = 0
        for b in [0, 1, 2, 3] + blocks:
            buf = obuf(b) * D
            if b == last_block:
                for osl in range(NOS):
                    co_thr += 1
                    sc.wait_ge(sems["co"], co_thr)
                    sc.dma_start(
                        out_d[b * P : (b + 1) * P, osl * NS : (osl + 1) * NS],
                        o_sb[:, buf + osl * NS : buf + (osl + 1) * NS],
                    ).then_inc(sems["osc"], 16)
                    n_osc += 1
            else:
                co_thr += 1
                sc.wait_ge(sems["co"], co_thr)
                sc.dma_start(
                    out_d[b * P : (b + 1) * P, :], o_sb[:, buf : buf + D]
                ).then_inc(sems["osc"], 16)
                n_osc += 1
        sc.wait_ge(sems["osc"], 16 * n_osc)

        # ---------------- vector engine: PSUM->SBUF copies -------------
        ve = nc.vector
        ve.memset(warm[:], 0.0).then_inc(sems["ws"], 1)
        for u, (kind, b, osl, banks) in enumerate(units):
            ve.wait_ge(sems["mm"], u + 1)
            if kind == "qp":
                p = osl
                last = None
                for ti in range(4):
                    last = ve.tensor_copy(
                        o_sb[:, ti * D + p * NS : ti * D + (p + 1) * NS],
                        ps[banks[ti]][:],
                    )
                last.then_inc(sems["cp"], 1)
                if p == NOS - 1:
                    ve.nop().then_inc(sems["co"], 4)
            elif kind == "pf":
                if obuf(b) < 5:  # dedicated buf 5: no reuse wait needed
                    ve.wait_ge(sems["osc"], 16)
                buf = obuf(b) * D
                last = None
                for j, o in enumerate(osl):
                    rows = slice(0, 64) if o % 2 == 0 else slice(64, P)
                    last = ve.tensor_copy(
                        o_sb[rows, buf + o * NS : buf + (o + 1) * NS],
                        ps[banks[j]][rows, :],
                    )
                last.then_inc(sems["cp"], 1)
                ve.nop().then_inc(sems["cf"], 1)
            else:  # bl
                bi = blocks.index(b)
                if osl == 0 and bi != 3:
                    # o_sb buf reuse: bufs cycle {0,1,2,4}; bi 0..2 reuse
                    # quad blocks 0..2's bufs (outs 1..3), bi 3 gets the
                    # fresh buf 4, bi>=4 reuses regular block bi-4's buf
                    # (out number bi+1 in scalar order). Both cases:
                    # wait for out bi+1.
                    ve.wait_ge(sems["osc"], 16 * (bi + 1))
                buf = obuf(b) * D
                last = ve.tensor_copy(
                    o_sb[:, buf + osl * NS : buf + (osl + 1) * NS], ps[banks[0]][:]
                )
                last.then_inc(sems["cp"], 1)
                if b == last_block:
                    ve.nop().then_inc(sems["co"], 1)
                elif osl == 3:
                    ve.nop().then_inc(sems["co"], 1)

        # ---------------- tensor engine: warmups + matmul stream -------
        te = nc.tensor
        te.wait_ge(sems["ws"], 1)
        for _ in range(N_WARM):
            te.matmul(
                ps[0][:64, :64], lhsT=warm[:, :64], rhs=warm[:, :64],
                start=True, stop=True,
            )
        last_use = {}
        for u, (kind, b, osl, banks) in enumerate(units):
            need = 0
            for bk in banks:
                if bk in last_use:
                    need = max(need, last_use[bk] + 1)
                last_use[bk] = u
            if need:
                te.wait_ge(sems["cp"], need)
            if kind == "qp":
                p = osl
                for kb in range(KB):
                    if kb % 4 == 0:
                        te.wait_ge(sems["w"], 16 * (p * 4 + kb // 4 + 1))
                    if p == 0:
                        if kb < 8:
                            if kb % 2 == 0:
                                te.wait_ge(sems["xqa"], 16 * (kb // 2 + 1))
                        elif kb % 2 == 0:
                            te.wait_ge(sems["xqb"], 16 * ((kb - 8) // 2 + 1))
                    for ti in range(4):
                        mm = te.matmul(
                            ps[banks[ti]][:],
                            lhsT=xs[:, xoff(ti, kb) : xoff(ti, kb) + P],
                            rhs=w_sl(wt, p, kb),
                            start=(kb == 0),
                            stop=(kb == KB - 1),
                        )
                mm.then_inc(sems["mm"], 1)
            elif kind == "pf":
                if osl[0] == 0:
                    te.wait_ge(sems["xb"], 16)
                for kb in range(KB):
                    for j, o in enumerate(osl):
                        dst = ps[banks[j]][:64, :] if o % 2 == 0 else ps[banks[j]][64:, :]
                        mm = te.matmul(
                            dst,
                            lhsT=xs[:, xoff(b, kb) : xoff(b, kb) + 64],
                            rhs=w_sl(wt, o, kb),
                            start=(kb == 0),
                            stop=(kb == KB - 1),
                        )
                mm.then_inc(sems["mm"], 1)
            else:  # bl
                if osl == 0:
                    te.wait_ge(sems["xb"], 16 * (border.index(b) + 1))
                for kb in range(KB):
                    mm = te.matmul(
                        ps[banks[0]][:],
                        lhsT=xs[:, xoff(b, kb) : xoff(b, kb) + P],
                        rhs=w_sl(wt, osl, kb),
                        start=(kb == 0),
                        stop=(kb == KB - 1),
                    )
                mm.then_inc(sems["mm"], 1)

        # ---------------- teardown: reset sems for the next execution --
        nc.all_engine_barrier()
        for s in sems.values():
            nc.gpsimd.sem_clear(s)
        nc.all_engine_barrier()

        nc.compile()
    return nc


def _get_nc(C, last_m):
    key = (C, last_m)
    if key not in _cache:
        _cache[key] = _build(C, last_m)
    return _cache[key]


def kernel(tokens, weight, exp_ids, _trace=False):
    _ensure_imports()
    from concourse.bass_utils import run_bass_kernel_spmd

    tokens = np.asarray(tokens)
    weight = np.asarray(weight)
    exp_ids = np.asarray(exp_ids)
    T = tokens.shape[0]

    order = np.argsort(exp_ids, kind="stable")
    counts = np.bincount(exp_ids, minlength=E)
    C = max(int(-(-counts.max() // P) * P), 640)
    TB = C // P

    rest = int(counts.max()) - (TB - 1) * P
    last_m = 64 if (TB >= 6 and rest <= 64) else 128
    FB = TB - 1 if last_m == 64 else None
    border = ([FB] if FB is not None else []) + [
        b for b in range(4, TB) if b != FB
    ]

    starts = np.zeros(E + 1, dtype=np.int64)
    np.cumsum(counts, out=starts[1:])

    tokens_c = tokens.astype(np.float16)
    weight_c = weight.astype(np.float16)

    in_maps = []
    for e in range(E):
        idx = order[starts[e] : starts[e + 1]]
        xt = np.zeros((D, C), dtype=np.float16)
        xt[:, : counts[e]] = tokens_c[idx].T
        xt3 = xt.reshape(KB, P, C)
        parts = [np.ascontiguousarray(xt3[:, :, :512].transpose(1, 0, 2)).reshape(P, -1)]
        for b in border:
            parts.append(
                np.ascontiguousarray(
                    xt3[:, :, b * P : (b + 1) * P].transpose(1, 0, 2)
                ).reshape(P, -1)
            )
        xs = np.concatenate(parts, axis=1)
        # weight os-major: [os 4][kb 16][128 p, 512] packed to [128, 4*16*512]
        w4 = weight_c[e].reshape(KB, P, NOS, NS)
        ww = np.ascontiguousarray(w4.transpose(1, 2, 0, 3)).reshape(P, -1)
        in_maps.append({"xs": xs, "w": ww})

    nc = _get_nc(C, last_m)
    res = run_bass_kernel_spmd(
        nc,
        in_maps,
        core_ids=list(range(E)),
        trace=_trace,
        trace_cores=list(range(E)) if _trace else None,
    )

    out = np.empty((T, D), dtype=np.float32)
    for e in range(E):
        idx = order[starts[e] : starts[e + 1]]
        out[idx] = res.results[e]["out"][: counts[e], :].astype(np.float32)
    if _trace:
        return out, res
    return out
